# revision 15
# baseline (speedup 1.0000x reference)
"""Bayesian MLP MC-sample kernel for one TRN2 chip (8 NeuronCores) - v2.

Problem: out[s, b, o] for S=32 MC samples of a 3-layer MLP
  dims 256 -> 512 -> 512 -> 64, batch B=2048,
  w_s = z_w[s] * exp(w_log_std) + w_mean   (per-sample reparameterized weights)
  h1 = tanh(x @ w0_s + b0_s); h2 = tanh(h1 @ w1_s + b1_s); out = h2 @ w2_s + b2_s

Sharding: MC-sample axis across the 8 cores (4 samples/core); x and the
mean/log_std parameters are replicated. No cross-core communication.

v2 design notes (from the v1 trace, 143.4us):
- All matmul operands bf16 (rel err ~4e-3 << 2e-2 gate). Same PE rate as
  f32r (1 col/cycle) but halves SBUF for h tiles, enabling deeper
  prefetch and 3 live h1 tiles.
- PSUM at bank granularity: [128,1024] 2-bank tiles, 3 rotating + warm
  + tail = 8 banks. Eviction per 2-bank tile on ACT (tanh+bias fused;
  L2 eviction is Copy+bias on ACT, out-DMA issued from the DVE queue so
  the sync ring never carries outputs).
- Startup: the v1 kernel's first real matmul was at 21.3us because the
  2.5MB of layer-0 params + x had to land first (sync ring ~290GB/s,
  first payload ~10.8us after the fixed ~8.5us engine preamble).
  v2 m-half-chunks the layer-0 params (dout 0:256 / 256:512) and
  interleaves x quarters so real MMs start ~16us, in DMA-arrival order.
- DMA order on the sync ring == consumption order; z transfers are
  half/k-chunked so DVE w-prep (w = z*sigma + mu) pipelines per chunk.
- Sample schedule: L0s0 L0s1 L0s2 L1s0 L0s3 L1s1 L2(s0,s1) L1s2 L1s3
  L2(s2,s3). Layer-2 pairs two samples' M=64 matmuls onto PE column
  strips via tile_position (0,0)/(0,64) - concurrent, halving L2 time.
- Bias data (z_b, b_mean, b_log_std: ~26KB) is host-packed into one
  [128, 52] f32 tensor in the exact SBUF layout (pure layout work);
  v1 spent a 7.6us DMA_DIRECT2D issue on a 4-byte-strided z_b pattern.
- ~34 bf16 warmup matmuls on zeroed tiles keep the PE HAM clock-gate
  warm (K=8/8, 2.4GHz) through the DMA-bound startup window.
"""

import ml_dtypes
import numpy as np

import concourse.bass as bass
import concourse.mybir as mybir
import concourse.tile as tile
from concourse import bacc
from concourse import bass_utils

F32 = mybir.dt.float32
BF16 = mybir.dt.bfloat16
MMDT = BF16
AF = mybir.ActivationFunctionType
ts = bass.ts

S = 32
B = 2048
DIMS = [256, 512, 512, 64]
NCORES = 8
SL = S // NCORES   # samples per core
NS = 512           # one PSUM bank of f32
NB = B // NS       # 4 n-slices
NK = [d // 128 for d in DIMS[:3]]        # k-chunks per layer: 2, 4, 4
NM = [max(1, d // 128) for d in DIMS[1:]]  # m-chunks: 4, 4, 1

# host-packed bias tensor layout: [128, BP_W] f32
# per layer: exp-able b_log_std block, b_mean block, z_b blocks
BC = [4, 4, 1]          # cols per item (dout/128; L2 uses 64 partitions)
BLS = [0, 24, 48]       # b_log_std col offset
BMN = [4, 28, 49]       # b_mean col offset
BZB = [8, 32, 50]       # z_b col offset (L0/L1: BC per sample; L2: 1 col/pair)
BP_W = 52

# knobs test.py may override before the first kernel() call
RUN_KWARGS: dict = {}
LAST_RESULT = None

_CACHE: dict = {}


def _build_nc():
    nc = bacc.Bacc("TRN2", target_bir_lowering=False)

    xT = nc.dram_tensor("xT", [DIMS[0], B], BF16, kind="ExternalInput")
    biaspack = nc.dram_tensor("biaspack", [128, BP_W], F32, kind="ExternalInput")
    w_mean, w_ls, z_w = [], [], []
    for li in range(3):
        din, dout = DIMS[li], DIMS[li + 1]
        w_mean.append(nc.dram_tensor(f"w_mean_{li}", [din, dout], F32, kind="ExternalInput"))
        w_ls.append(nc.dram_tensor(f"w_log_std_{li}", [din, dout], F32, kind="ExternalInput"))
        z_w.append(nc.dram_tensor(f"z_w_{li}", [SL, din, dout], BF16, kind="ExternalInput"))
    # pair-packed output: row 0-63 = even sample, 64-127 = odd sample of pair j
    out_d = nc.dram_tensor("out", [SL // 2, 2 * DIMS[3], B], F32, kind="ExternalOutput")

    with tile.TileContext(nc) as tc:
        with (
            tc.tile_pool(name="const", bufs=1) as cpool,
            tc.tile_pool(name="z", bufs=4) as zpool,
            tc.tile_pool(name="w0", bufs=2) as w0p,
            tc.tile_pool(name="w1", bufs=2) as w1p,
            tc.tile_pool(name="w2", bufs=2) as w2p,
            tc.tile_pool(name="h1", bufs=3) as h1p,
            tc.tile_pool(name="h2", bufs=2) as h2p,
            tc.tile_pool(name="osb", bufs=4) as opool,
            tc.tile_pool(name="ps", bufs=3, space="PSUM") as pspool,
            tc.tile_pool(name="pst", bufs=1, space="PSUM") as pstp,
        ):
            hwd = nc.sync      # input DMA ring (issue order == priority)
            # output DMAs also ride the sync ring, but are EMITTED after
            # every input dma_start so they can never head-of-line-block a
            # z prefetch (the ring executes in emission order)
            odma = nc.sync

            sigma = [None] * 3   # exp(w_log_std), f32, [128, nk, dout]
            mean = [None] * 3    # w_mean, f32, [128, nk, dout]
            w_tiles = {}
            h1_tiles = {}
            h2_tiles = {}

            # ---- bias pack ----
            bp_t = cpool.tile([128, BP_W], F32, tag="bp")

            def bias_ap(li, s):
                c = BC[li]
                return bp_t[:, BZB[li] + c * s : BZB[li] + c * (s + 1)]

            def emit_bias_exp(li):
                c = BC[li]
                sl_ = bp_t[:, BLS[li] : BLS[li] + c]
                nc.scalar.activation(sl_, sl_, AF.Exp)

            def emit_bias_prep(li, s):
                c = BC[li]
                col = bias_ap(li, s)
                nc.vector.tensor_mul(col, col, bp_t[:, BLS[li] : BLS[li] + c])
                nc.vector.tensor_add(col, col, bp_t[:, BMN[li] : BMN[li] + c])

            # ---- generic w-prep: w = z * sigma + mean, chunked DVE ----
            z_tiles = {}

            def emit_zdma(li, s, half_dma=False):
                nk, dout = NK[li], DIMS[li + 1]
                zt = zpool.tile([128, nk, dout], BF16, tag="z")
                zsrc = z_w[li][s].rearrange("(k p) d -> p k d", p=128)
                if half_dma and nk >= 2:
                    h = nk // 2
                    hwd.dma_start(zt[:, 0:h, :], zsrc[:, 0:h, :])
                    hwd.dma_start(zt[:, h:nk, :], zsrc[:, h:nk, :])
                else:
                    hwd.dma_start(zt[:], zsrc)
                z_tiles[(li, s)] = zt

            def emit_wprep(li, s, wpool, half_dma=False):
                nk, dout = NK[li], DIMS[li + 1]
                if (li, s) not in z_tiles:
                    emit_zdma(li, s, half_dma=half_dma)
                zt = z_tiles.pop((li, s))
                wt = wpool.tile([128, nk, dout], MMDT, tag=f"w{li}")
                for k in range(nk):
                    nc.vector.tensor_mul(zt[:, k, :], zt[:, k, :], sigma[li][:, k, :])
                    nc.vector.tensor_add(wt[:, k, :], zt[:, k, :], mean[li][:, k, :])
                w_tiles[(li, s)] = wt
                if li < 2:
                    emit_bias_prep(li, s)

            # ---- layer 0/1 matmuls: per-m, per-n-pair 2-bank psum tiles ----
            def emit_l01_mms(li, s, korder=False):
                nk = NK[li]
                wt = w_tiles.pop((li, s))
                bt = bias_ap(li, s)
                src = xbf if li == 0 else h1_tiles[s]
                if li == 0:
                    dst = h1p.tile([128, NM[0], B], MMDT, tag="h1")
                    h1_tiles[s] = dst
                else:
                    dst = h2p.tile([128, NM[1], B], MMDT, tag="h2")
                    h2_tiles[s] = dst
                for m in range(NM[li]):
                    for npair in range(2):
                        ps = pspool.tile([128, 2 * NS], F32, tag="ps")
                        if korder:
                            # k-outer within the tile: the first matmul needs
                            # only w k-chunk 0 (prep still streaming in)
                            kn = [(k, nn) for k in range(nk) for nn in range(2)]
                        else:
                            kn = [(k, nn) for nn in range(2) for k in range(nk)]
                        for k, nn in kn:
                            n = npair * 2 + nn
                            nc.tensor.matmul(
                                ps[:, ts(nn, NS)],
                                wt[:, k, ts(m, 128)],
                                src[:, k, ts(n, NS)],
                                start=(k == 0),
                                stop=(k == nk - 1),
                            )
                        nc.scalar.activation(
                            dst[:, m, ts(npair, 2 * NS)], ps[:],
                            AF.Tanh, bias=bt[:, m : m + 1],
                        )
                if li == 1:
                    h1_tiles.pop(s, None)

            # ---- layer 2: two samples packed on PE column strips ----
            out_dmas = []  # (dst_ap, src_ap): emitted on sync after all inputs

            def emit_l2_pair(j, tail=False):
                sa, sb = 2 * j, 2 * j + 1
                wa = w_tiles.pop((2, sa))
                wb = w_tiles.pop((2, sb))
                ha = h2_tiles.pop(sa)
                hb = h2_tiles.pop(sb)
                bt = bias_ap(2, j)  # [128,1]: sa bias on parts 0-63, sb on 64-127
                nk = NK[2]

                def strip_mms(psl, n):
                    for k in range(nk):
                        nc.tensor.matmul(
                            psl[0:64, ts(n % 2, NS)], wa[:, k, :], ha[:, k, ts(n, NS)],
                            start=(k == 0), stop=(k == nk - 1), tile_position=(0, 0),
                        )
                        nc.tensor.matmul(
                            psl[64:128, ts(n % 2, NS)], wb[:, k, :], hb[:, k, ts(n, NS)],
                            start=(k == 0), stop=(k == nk - 1), tile_position=(0, 64),
                        )

                if tail:
                    # fine-grained: per-bank psum per n-slice so output DMA
                    # starts ~1us into the pair and overlaps the matmuls
                    for n in range(NB):
                        pool, ptag = (pstp, "pst") if n == 0 else (pspool, "ps")
                        pt = pool.tile([128, 2 * NS], F32, tag=ptag, name=f"pt{n}")
                        for k in range(nk):
                            nc.tensor.matmul(
                                pt[0:64, 0:NS], wa[:, k, :], ha[:, k, ts(n, NS)],
                                start=(k == 0), stop=(k == nk - 1), tile_position=(0, 0),
                            )
                            nc.tensor.matmul(
                                pt[64:128, 0:NS], wb[:, k, :], hb[:, k, ts(n, NS)],
                                start=(k == 0), stop=(k == nk - 1), tile_position=(0, 64),
                            )
                        osb = opool.tile([128, 2 * NS], F32, tag="osb")
                        nc.scalar.activation(
                            osb[:, 0:NS], pt[:, 0:NS], AF.Identity, bias=bt[:, 0:1]
                        )
                        out_dmas.append((out_d[j][:, ts(n, NS)], osb[:, 0:NS]))
                else:
                    for npair in range(2):
                        ps = pspool.tile([128, 2 * NS], F32, tag="ps")
                        strip_mms(ps, npair * 2)
                        strip_mms(ps, npair * 2 + 1)
                        osb = opool.tile([128, 2 * NS], F32, tag="osb")
                        nc.scalar.activation(osb[:], ps[:], AF.Identity, bias=bt[:, 0:1])
                        out_dmas.append((out_d[j][:, ts(npair, 2 * NS)], osb[:]))

            # ================= PE warm-up =================
            # HAM gates the PE clock to 1.2GHz until ~3.4us of sustained
            # activity; dummy bf16 matmuls cover the DMA-bound startup.
            warm_w = cpool.tile([128, 128], BF16, tag="warm_w")
            warm_x = cpool.tile([128, NS], BF16, tag="warm_x")
            nc.gpsimd.memset(warm_w[:], 0.0)
            nc.gpsimd.memset(warm_x[:], 0.0)

            # ================= startup: layer-0 sample-0, m-half chunked =====
            # sync ring order (== arrival order):
            #   s0A(ls,z,mn) xq0 bias xq1 s0B xq2 xq3 z01 z02 s1(ls,z,mn) z03
            #   z11 s2consts z20 z21 z12 z13 z22 z23
            HD = 256  # dout half for layer-0 param chunks
            sg0 = cpool.tile([128, NK[0], DIMS[1]], F32, tag="sigma0")
            zt0 = zpool.tile([128, NK[0], DIMS[1]], BF16, tag="z")
            mn0 = cpool.tile([128, NK[0], DIMS[1]], F32, tag="mean0")
            wt0 = w0p.tile([128, NK[0], DIMS[1]], MMDT, tag="w0")
            sigma[0], mean[0] = sg0, mn0
            sg0_src = w_ls[0][:].rearrange("(k p) d -> p k d", p=128)
            z0_src = z_w[0][0].rearrange("(k p) d -> p k d", p=128)
            mn0_src = w_mean[0][:].rearrange("(k p) d -> p k d", p=128)
            xbf = cpool.tile([128, NK[0], B], MMDT, tag="xbf")
            x_src = xT[:].rearrange("(k p) n -> p k n", p=128)

            def l0_param_half(h):
                d0 = ts(h, HD)
                hwd.dma_start(sg0[:, :, d0], sg0_src[:, :, d0])
                nc.scalar.activation(sg0[:, :, d0], sg0[:, :, d0], AF.Exp)
                hwd.dma_start(zt0[:, :, d0], z0_src[:, :, d0])
                hwd.dma_start(mn0[:, :, d0], mn0_src[:, :, d0])
                nc.vector.tensor_mul(zt0[:, :, d0], zt0[:, :, d0], sg0[:, :, d0])
                nc.vector.tensor_add(wt0[:, :, d0], zt0[:, :, d0], mn0[:, :, d0])

            # bias + x halves ride the scalar HWDGE ring: a second descriptor
            # queue issuing in parallel with the sync ring (the two rings'
            # aggregate hits ~400GB/s vs ~280 for one). Only these three:
            # more would eat the SHARED 8-transfer outstanding window and a
            # slot-starved issue head-of-line-blocks the whole ACT queue.
            # x arrives bf16 from the host - straight into the matmul tile.
            nc.scalar.dma_start(bp_t[:], biaspack[:])
            nc.scalar.dma_start(xbf[:, :, 0 : 2 * NS], x_src[:, :, 0 : 2 * NS])
            nc.scalar.dma_start(xbf[:, :, 2 * NS : B], x_src[:, :, 2 * NS : B])
            l0_param_half(0)
            for li in range(3):
                emit_bias_exp(li)
            emit_bias_prep(0, 0)
            l0_param_half(1)

            # layer-1 params next on the ring (k-triples); their exps and
            # DVE prep are placed further down, into engine-queue slack
            sg1 = cpool.tile([128, NK[1], DIMS[2]], F32, tag="sigma1")
            zt1 = zpool.tile([128, NK[1], DIMS[2]], BF16, tag="z")
            mn1 = cpool.tile([128, NK[1], DIMS[2]], F32, tag="mean1")
            sigma[1], mean[1] = sg1, mn1
            sg1_src = w_ls[1][:].rearrange("(k p) d -> p k d", p=128)
            z1_src = z_w[1][0].rearrange("(k p) d -> p k d", p=128)
            mn1_src = w_mean[1][:].rearrange("(k p) d -> p k d", p=128)
            for k in range(NK[1]):
                hwd.dma_start(sg1[:, k, :], sg1_src[:, k, :])
                hwd.dma_start(zt1[:, k, :], z1_src[:, k, :])
                hwd.dma_start(mn1[:, k, :], mn1_src[:, k, :])

            # L0 s0 matmuls in DMA-arrival order; [128,1024] tiles hold two
            # (m, n) banks, each evicted separately (different m -> different
            # h1 slice).
            dst00 = h1p.tile([128, NM[0], B], MMDT, tag="h1")
            h1_tiles[0] = dst00
            bt00 = bias_ap(0, 0)
            groups = [
                (0, 0), (1, 0), (0, 1), (1, 1),  # half A, q0/q1
                (2, 0), (3, 0), (2, 1), (3, 1),  # half B
                (0, 2), (1, 2), (2, 2), (3, 2),  # q2
                (0, 3), (1, 3), (2, 3), (3, 3),  # q3
            ]
            for gi in range(0, len(groups), 2):
                if gi >= 8 and (gi - 8) // 2 < NK[1]:
                    # sigma1 exp k-chunks ride ACT slots between evictions
                    k = (gi - 8) // 2
                    nc.scalar.activation(sg1[:, k, :], sg1[:, k, :], AF.Exp)
                ps = pspool.tile([128, 2 * NS], F32, tag="ps")
                if gi == 0:
                    # warmup dummies share this tile; the first real matmul
                    # has start=True which resets the bank
                    for _ in range(24):
                        nc.tensor.matmul(
                            ps[:, 0:NS], warm_w[:], warm_x[:], start=True, stop=True
                        )
                for half, (m, n) in enumerate(groups[gi : gi + 2]):
                    for k in range(NK[0]):
                        nc.tensor.matmul(
                            ps[:, ts(half, NS)],
                            wt0[:, k, ts(m, 128)],
                            xbf[:, k, ts(n, NS)],
                            start=(k == 0),
                            stop=(k == NK[0] - 1),
                        )
                    nc.scalar.activation(
                        dst00[:, m, ts(n, NS)], ps[:, ts(half, NS)],
                        AF.Tanh, bias=bt00[:, m : m + 1],
                    )
            w_tiles[(0, 0)] = None  # consumed above

            # ---- L0 s1/s2 z prefetch + DVE prep BEFORE the w1s0 prep so
            # the (late-exp'd) sigma1 muls never block them ----
            emit_zdma(0, 1, half_dma=True)
            emit_zdma(0, 2, half_dma=True)
            emit_wprep(0, 1, w0p)
            emit_l01_mms(0, 1)
            emit_wprep(0, 2, w0p)

            wt1 = w1p.tile([128, NK[1], DIMS[2]], MMDT, tag="w1")
            for k in range(NK[1]):
                nc.vector.tensor_mul(zt1[:, k, :], zt1[:, k, :], sg1[:, k, :])
                nc.vector.tensor_add(wt1[:, k, :], zt1[:, k, :], mn1[:, k, :])
            w_tiles[(1, 0)] = wt1
            emit_bias_prep(1, 0)

            emit_l01_mms(1, 0, korder=True)

            # ---- L0 s2 / L1 s1 / L0 s3 interleaved so the ACT eviction
            # stream never saturates across consecutive L0 samples ----
            emit_l01_mms(0, 2)
            emit_wprep(1, 1, w1p, half_dma=True)
            emit_l01_mms(1, 1)
            emit_wprep(0, 3, w0p, half_dma=True)
            emit_l01_mms(0, 3)

            # ---- layer-2 consts + pair 0 preps (emitted before L2 MMs) ----
            sg2 = cpool.tile([128, NK[2], DIMS[3]], F32, tag="sigma2")
            mn2 = cpool.tile([128, NK[2], DIMS[3]], F32, tag="mean2")
            sigma[2], mean[2] = sg2, mn2
            hwd.dma_start(sg2[:], w_ls[2][:].rearrange("(k p) d -> p k d", p=128))
            nc.scalar.activation(sg2[:], sg2[:], AF.Exp)
            hwd.dma_start(mn2[:], w_mean[2][:].rearrange("(k p) d -> p k d", p=128))
            emit_wprep(2, 0, w2p)
            emit_wprep(2, 1, w2p)
            emit_bias_prep(2, 0)
            emit_bias_prep(2, 1)
            emit_l2_pair(0)

            # ---- L1 s2, s3; L2 pair 1 preps run ahead on DVE ----
            emit_wprep(1, 2, w1p, half_dma=True)
            emit_l01_mms(1, 2)
            emit_wprep(1, 3, w1p, half_dma=True)
            emit_wprep(2, 2, w2p)
            emit_wprep(2, 3, w2p)

            def flush_out_dmas():
                for dst, src in out_dmas:
                    odma.dma_start(dst, src)
                out_dmas.clear()

            # all input dma_starts are emitted; pair-0 outputs can now ride
            # the sync ring without blocking any prefetch
            flush_out_dmas()
            emit_l01_mms(1, 3)
            emit_l2_pair(1, tail=True)
            flush_out_dmas()

    nc.compile()
    return nc


def _get_nc():
    if "nc" not in _CACHE:
        _CACHE["nc"] = _build_nc()
    return _CACHE["nc"]


def _pack_bias(inp, s0):
    """Pack z_b / b_mean / b_log_std for samples [s0, s0+SL) into the
    [128, BP_W] SBUF-layout tensor (pure layout work)."""
    bp = np.zeros((128, BP_W), np.float32)
    for li in (0, 1):
        c = BC[li]
        bp[:, BLS[li] : BLS[li] + c] = inp[f"b_log_std_{li}"].reshape(c, 128).T
        bp[:, BMN[li] : BMN[li] + c] = inp[f"b_mean_{li}"].reshape(c, 128).T
        zb = inp[f"z_b_{li}"][s0 : s0 + SL, 0, :]
        for s_ in range(SL):
            bp[:, BZB[li] + c * s_ : BZB[li] + c * (s_ + 1)] = zb[s_].reshape(c, 128).T
    # layer 2: 64 partitions, duplicated for the column-strip sample pairing
    for half in (slice(0, 64), slice(64, 128)):
        bp[half, BLS[2]] = inp["b_log_std_2"]
        bp[half, BMN[2]] = inp["b_mean_2"]
    zb2 = inp["z_b_2"][s0 : s0 + SL, 0, :]
    bp[0:64, BZB[2]] = zb2[0]
    bp[64:128, BZB[2]] = zb2[1]
    bp[0:64, BZB[2] + 1] = zb2[2]
    bp[64:128, BZB[2] + 1] = zb2[3]
    return bp


def kernel(**inputs) -> np.ndarray:
    global LAST_RESULT
    nc = _get_nc()
    inp = {k: np.asarray(v, dtype=np.float32) for k, v in inputs.items()}

    xT = np.ascontiguousarray(inp["x"].T).astype(ml_dtypes.bfloat16)
    in_maps = []
    for c in range(NCORES):
        sl = slice(c * SL, (c + 1) * SL)
        m = {"xT": xT, "biaspack": _pack_bias(inp, c * SL)}
        for li in range(3):
            m[f"w_mean_{li}"] = inp[f"w_mean_{li}"]
            m[f"w_log_std_{li}"] = inp[f"w_log_std_{li}"]
            m[f"z_w_{li}"] = np.ascontiguousarray(
                inp[f"z_w_{li}"][sl]
            ).astype(ml_dtypes.bfloat16)
        in_maps.append(m)

    res = bass_utils.run_bass_kernel_spmd(
        nc, in_maps, core_ids=list(range(NCORES)), **RUN_KWARGS
    )
    LAST_RESULT = res
    # per-core out: [SL//2, 128, B] with pair j = (sample 2j on rows 0:64,
    # sample 2j+1 on rows 64:128) -> [SL, 64, B]
    full = np.concatenate(
        [
            res.results[c]["out"].reshape(SL, DIMS[3], B)
            for c in range(NCORES)
        ],
        axis=0,
    )
    return np.ascontiguousarray(full.transpose(0, 2, 1)).astype(np.float32)
